# revision 2
# baseline (speedup 1.0000x reference)
"""Trainium2 Bass kernel for nn_NeighSuperpixelAgg — v2.

Per image: v = x@WvT+bv; o = NATTEN-AV(attn, v, 7x7 clamped); y = o@WpT+bp.
Data-parallel over batch B=8 (one image per core).

Engine split (per output row):
  DVE : 7 kj-shifted products, all-bf16 packed-innermost -> 2x mode.
  PE  : x transposes, v proj, 49 identity-stationary accumulate matmuls
        (PSUM fp32 does ALL (ki,kj) summation), o transposes, y proj.
  ACT : every PSUM -> SBUF bf16 copy.
  Pool: optionally one product (knob N_POOL_PRODUCTS).

Layout trick: channels permuted c' = (d2, h, pair) with d = 2*d2+pair,
folded into Wv columns / Wp rows on the host.  attn is shipped already
rearranged to [i, j, kj, ki, h, pair] bf16 with the value duplicated over
pair, so the product's in1 AP has a packed (stride-1, n>=2) innermost dim
-> DVE 2x perf mode for every product.

Edges (cols 0-2, 125-127) recomputed rows-on-partitions after the main
loop; 3x3 corner blocks share one clamped 7x7 window per block -> DVE
product vs host-expanded attn + ones-matmul per pixel.
"""

import numpy as np

import concourse.bass as bass
import concourse.bacc as bacc
import concourse.tile as tile
from concourse import mybir
from concourse.masks import make_identity

C = 256
NH = 8
HD = 32
K = 7
KK = 49
R = 16  # ring depth; stored doubled (2R slots) so ki windows never wrap
G = 4  # rows per DMA/attn group
FP = mybir.dt.float32
BF = mybir.dt.bfloat16

N_POOL_PRODUCTS = 0  # 0..1: products given to gpsimd instead of DVE


def build_nc(H: int = 128, W: int = 128) -> bass.Bass:
    assert W == 128 and H % G == 0 and H >= 2 * G + K
    HW = H * W
    nc = bacc.Bacc()

    x_d = nc.declare_dram_parameter("xbf", [HW, C], BF, isOutput=False)
    attn_d = nc.declare_dram_parameter("attnp", [HW, K * K * NH * 2], BF, isOutput=False)
    ae_d = nc.declare_dram_parameter("aedge", [H, 6 * KK * NH * 2], BF, isOutput=False)
    acorn_d = nc.declare_dram_parameter("acorn", [KK, 36 * NH], BF, isOutput=False)
    wvt_d = nc.declare_dram_parameter("wvt", [C, C], BF, isOutput=False)
    bv_d = nc.declare_dram_parameter("bv", [1, C], BF, isOutput=False)
    wpt_d = nc.declare_dram_parameter("wpt", [C, C], BF, isOutput=False)
    wpt0_d = nc.declare_dram_parameter("wpt0", [C, C], BF, isOutput=False)
    bp_d = nc.declare_dram_parameter("bp", [1, C], BF, isOutput=False)
    out_d = nc.declare_dram_parameter("out", [HW, C], BF, isOutput=True)

    NG = H // G

    with tile.TileContext(nc) as tc:
        with (
            tc.tile_pool(name="singles", bufs=1) as singles,
            tc.tile_pool(name="ps_v", bufs=2, space="PSUM") as ps_v,
            tc.tile_pool(name="ps_a", bufs=2, space="PSUM") as ps_a,
            tc.tile_pool(name="ps_t", bufs=2, space="PSUM") as ps_t,
            tc.tile_pool(name="ps_y", bufs=2, space="PSUM") as ps_y,
        ):
            # ---------------- persistent SBUF ----------------
            wvt_sb = singles.tile([128, 2 * C], BF)  # [ci_half, (half, c')]
            nc.sync.dma_start(wvt_sb[:, 0:C], wvt_d[:][0:128, :])
            nc.sync.dma_start(wvt_sb[:, C : 2 * C], wvt_d[:][128:256, :])
            wpt_sb = singles.tile([128, 2 * C], BF)  # [c'_half, (half, c'')]
            nc.sync.dma_start(wpt_sb[:, 0:C], wpt_d[:][0:128, :])
            nc.sync.dma_start(wpt_sb[:, C : 2 * C], wpt_d[:][128:256, :])
            wpt0_sb = singles.tile([128, 2 * C], BF)  # unpermuted (corners)
            nc.sync.dma_start(wpt0_sb[:, 0:C], wpt0_d[:][0:128, :])
            nc.sync.dma_start(wpt0_sb[:, C : 2 * C], wpt0_d[:][128:256, :])
            bv_sb = singles.tile([1, C], BF)
            nc.sync.dma_start(bv_sb, bv_d[:])
            bp_sb = singles.tile([1, C], BF)
            nc.sync.dma_start(bp_sb, bp_d[:])

            ones1 = singles.tile([1, 128], BF)
            nc.vector.memset(ones1, 1.0)
            ones49 = singles.tile([KK, 1], BF)
            nc.vector.memset(ones49, 1.0)
            ident = singles.tile([128, 128], BF)
            make_identity(nc, ident)

            # Pre-touch weight DMAs with throwaway matmuls (one fresh
            # DMA-queue wait max per later matmul).
            dps = ps_t.tile([128, 128], FP, name="dps", tag="tp")
            for t in (
                wvt_sb[:, 0:C], wvt_sb[:, C : 2 * C],
                wpt_sb[:, 0:C], wpt_sb[:, C : 2 * C],
                bv_sb, bp_sb,
            ):
                nc.tensor.matmul(
                    dps, t[0:1, 0:128], t[0:1, 0:128], start=True, stop=True
                )

            # edge-column strip of every v row: [i, (14 cols, c')] bf16
            v_edge = singles.tile([H, 14 * C], BF)
            nc.vector.memset(v_edge, 0.0)

            state = {}

            # ---------------- phase A: v projection ----------------------
            def proj_load(r0: int):
                xrp, vsp = state["xrp"], state["vsp"]
                xr = xrp.tile([W, G * C], BF, name="xr")
                nc.sync.dma_start(
                    xr.rearrange("p (g c) -> p g c", g=G),
                    x_d[:]
                    .rearrange("(i j) c -> j i c", j=W)[:, r0 : r0 + G, :],
                )
                v4 = vsp.tile([W, G * C], BF, name="v4")
                state["proj_cur"][r0] = (xr, v4)

            def proj_row(r0: int, g: int):
                xtp = state["xtp"]
                xr, v4 = state["proj_cur"][r0]
                if True:
                    # transpose x row -> [ci, j] halves
                    xt = xtp.tile([128, 2 * W], BF, name="xt")
                    for hf in range(2):
                        tp = ps_t.tile([128, W], BF, name="xtp_ps", tag="tp")
                        nc.tensor.transpose(
                            tp, xr[:, g * C + hf * 128 : g * C + (hf + 1) * 128],
                            ident,
                        )
                        nc.scalar.copy(xt[:, hf * W : (hf + 1) * W], tp)
                    v_ps = ps_v.tile([W, C], FP, name="v_ps", tag="v")
                    nc.tensor.matmul(
                        v_ps, xt[:, 0:W], wvt_sb[:, 0:C], start=True, stop=False
                    )
                    nc.tensor.matmul(
                        v_ps, xt[:, W : 2 * W], wvt_sb[:, C : 2 * C],
                        start=False, stop=False,
                    )
                    nc.tensor.matmul(v_ps, ones1, bv_sb, start=False, stop=True)
                    nc.scalar.copy(v4[:, g * C : (g + 1) * C], v_ps)
                    # edge strips (consumed after the barrier; Pool/SWDGE
                    # queue keeps them off the critical path)
                    vev = v_edge.rearrange("p (cc c) -> p cc c", cc=14)
                    nc.gpsimd.dma_start(
                        vev[r0 + g : r0 + g + 1, 0:7, :],
                        v4[0:7, g * C : (g + 1) * C],
                    )
                    nc.gpsimd.dma_start(
                        vev[r0 + g : r0 + g + 1, 7:14, :],
                        v4[W - 7 : W, g * C : (g + 1) * C],
                    )
            def proj_ring(r0: int, h0: int, h1: int):
                # ring scatter for group rows [h0, h1): one DMA per kj
                _, v4 = state["proj_cur"][r0]
                if h1 == G:
                    state["proj_cur"].pop(r0)
                s0 = r0 % R
                vrw = state["vrw"]
                for kj in range(K):
                    jlo = max(0, 3 - kj)
                    jhi = min(W, W + 3 - kj)
                    src = v4[jlo + kj - 3 : jhi + kj - 3, :]
                    for a0, a1 in ((h0, min(h1, R - s0)), (max(h0, R - s0), h1)):
                        if a0 >= a1:
                            continue
                        sl = (s0 + a0) % R
                        nc.sync.dma_start(
                            vrw[jlo:jhi, kj, :, sl * C : (sl + a1 - a0) * C],
                            src[:, a0 * C : a1 * C]
                            .rearrange("p (a f) -> p a f", a=1)
                            .broadcast_to([jhi - jlo, 2, (a1 - a0) * C]),
                        )

            # ---------------- phase B+D: aggregation + out proj ----------
            # accumulate ki-slices pairwise into a [W, 512] PSUM (halves
            # the PE instruction count); Pool folds the two halves.
            def agg_core(i: int, pool_pt):
                prodp, outp, vrr = (
                    state["prodp"], state["outp"], state["vrr"]
                )
                av, g = state["av"], i % G
                si = min(max(i - 3, 0), H - K)
                s0 = si % R
                acc2 = ps_a.tile([W, 2 * C], FP, name="acc", tag="acc")
                first = True
                for kj in range(K):
                    if kj == K - 1 and pool_pt is not None:
                        pt = pool_pt
                    else:
                        in0 = vrr[:, kj, s0 : s0 + K, :].rearrange(
                            "p s (d2 hp) -> p s d2 hp", d2=16
                        )
                        in1 = (
                            av[:, g, kj, :, :]
                            .rearrange("p ki (a hp) -> p ki a hp", a=1)
                            .broadcast_to([W, K, 16, NH * 2])
                        )
                        pt = prodp.tile([W, K * C], BF, name="pt")
                        nc.vector.tensor_tensor(
                            pt.rearrange(
                                "p (s d2 hp) -> p s d2 hp", s=K, d2=16
                            ),
                            in0, in1, mybir.AluOpType.mult,
                        )
                    for k0 in range(4):  # ki pairs (0,1)(2,3)(4,5) + ki 6
                        n = 2 * C if k0 < 3 else C
                        nc.tensor.matmul(
                            acc2[:, 0:n],
                            ident,
                            pt[:, k0 * 2 * C : k0 * 2 * C + n],
                            start=first,
                            stop=(kj == K - 1 and k0 == 3),
                        )
                        first = False
                a2_sb = outp.tile([W, 2 * C], BF, name="a2_sb", tag="o2")
                nc.scalar.copy(a2_sb, acc2)
                o_sb = outp.tile([W, C], BF, name="o_sb", tag="o")
                nc.gpsimd.tensor_tensor(
                    o_sb, a2_sb[:, 0:C], a2_sb[:, C : 2 * C],
                    mybir.AluOpType.add,
                )
                return o_sb

            def agg_tail(o_sb, y4, g: int):
                otp = state["otp"]
                ot = otp.tile([128, 2 * W], BF, name="ot")
                for hf in range(2):
                    tp = ps_t.tile([128, W], BF, name="tp", tag="tp")
                    nc.tensor.transpose(
                        tp, o_sb[:, hf * 128 : (hf + 1) * 128], ident
                    )
                    nc.scalar.copy(ot[:, hf * W : (hf + 1) * W], tp)
                y_ps = ps_y.tile([W, C], FP, name="y_ps", tag="y")
                nc.tensor.matmul(
                    y_ps, ot[:, 0:W], wpt_sb[:, 0:C], start=True, stop=False
                )
                nc.tensor.matmul(
                    y_ps, ot[:, W : 2 * W], wpt_sb[:, C : 2 * C],
                    start=False, stop=False,
                )
                nc.tensor.matmul(y_ps, ones1, bp_sb, start=False, stop=True)
                nc.scalar.copy(y4[:, g * C : (g + 1) * C], y_ps)

            def flush_y4(y4, r0: int):
                nc.sync.dma_start(
                    out_d[:]
                    .rearrange("(i j) c -> j i c", j=W)[:, r0 : r0 + G, :],
                    y4.rearrange("p (g c) -> p g c", g=G),
                )

            def prefetch_attn(r0: int):
                attnp = state["attnp"]
                a_sb = attnp.tile([W, G * KK * NH * 2], BF, name="a_sb")
                nc.sync.dma_start(
                    a_sb.rearrange("p (g f) -> p g f", g=G),
                    attn_d[:]
                    .rearrange("(i j) f -> j i f", j=W)[:, r0 : r0 + G, :],
                )
                state["a_pre"][r0] = a_sb

            def emit_agg_group(r0: int, proj_r0=None):
                y4p = state["y4p"]
                a_sb = state["a_pre"].pop(r0)
                state["av"] = a_sb.rearrange(
                    "p (g kj ki hp) -> p g kj ki hp", g=G, kj=K, ki=K
                )
                y4 = y4p.tile([W, G * C], BF, name="y4")
                for g in range(G):
                    o_sb = agg_core(r0 + g, None)
                    # run the previous row's projection tail one row late so
                    # its cross-engine hops hide under this row's accums
                    if state["pend"] is not None:
                        po, py4, pg, pr0 = state["pend"]
                        agg_tail(po, py4, pg)
                        if pg == G - 1:
                            flush_y4(py4, pr0)
                    state["pend"] = (o_sb, y4, g, r0)
                    # smear the lookahead projection across rows; ring DMAs
                    # (which the next group's products serialize behind) go
                    # out in two halves as early as WAR deps allow
                    if proj_r0 is not None:
                        for pg in ([0], [1, 2], [3], [])[g]:
                            proj_row(proj_r0, pg)
                        if g == 1:
                            proj_ring(proj_r0, 0, 2)
                        elif g == 2:
                            proj_ring(proj_r0, 2, G)

            def flush_agg():
                if state["pend"] is not None:
                    o_sb, y4, g, r0 = state["pend"]
                    agg_tail(o_sb, y4, g)
                    if g == G - 1:
                        flush_y4(y4, r0)
                    state["pend"] = None

            state["pend"] = None
            state["a_pre"] = {}
            state["proj_cur"] = {}

            # ---------------- phase C: edge columns + corners ------------
            def emit_edges():
                ae, vew, prodp2, edgeo = (
                    state["ae"], state["vew"], state["prodp2"], state["edgeo"]
                )
                nc.sync.dma_start(ae, ae_d[:])
                vew4 = vew.rearrange(
                    "p (ki cc c) -> p ki cc c", ki=K, cc=K
                )
                vev = v_edge.rearrange("p (cc c) -> p cc c", cc=14)
                aev = ae.rearrange(
                    "p (q kc hp) -> p q kc hp", q=6, kc=KK
                )
                o3 = out_d[:].rearrange("(i j) c -> i j c", j=W)
                for side in range(2):
                    for ki in range(K):
                        ilo = max(0, 3 - ki)
                        ihi = min(H, H + 3 - ki)
                        nc.sync.dma_start(
                            vew4[ilo:ihi, ki, :, :],
                            vev[
                                ilo + ki - 3 : ihi + ki - 3,
                                side * K : (side + 1) * K,
                                :,
                            ],
                        )
                    for col in range(3):
                        q = side * 3 + col
                        j0 = col if side == 0 else W - 3 + col
                        pe = prodp2.tile([H, KK * C], BF, name="pe")
                        pev = pe.rearrange(
                            "p (kc d2 hp) -> p kc d2 hp", kc=KK, d2=16
                        )
                        in0 = vew.rearrange(
                            "p (kc d2 hp) -> p kc d2 hp", kc=KK, d2=16
                        )
                        in1 = (
                            aev[:, q, :, :]
                            .rearrange("p kc (a hp) -> p kc a hp", a=1)
                            .broadcast_to([H, KK, 16, NH * 2])
                        )
                        nc.vector.tensor_tensor(
                            pev, in0, in1, mybir.AluOpType.mult
                        )
                        acc = ps_a.tile([H, C], FP, name="acc_e", tag="acc")
                        for kc in range(KK):
                            nc.tensor.matmul(
                                acc,
                                ident,
                                pe[:, kc * C : (kc + 1) * C],
                                start=(kc == 0),
                                stop=(kc == KK - 1),
                            )
                        oe = edgeo.tile([H, C], BF, name="oe")
                        nc.scalar.copy(oe, acc)
                        ot = edgeo.tile([128, 2 * H], BF, name="ot_e")
                        for hf in range(2):
                            tp = ps_t.tile([128, H], BF, name="tp_e", tag="tp")
                            nc.tensor.transpose(
                                tp, oe[:, hf * 128 : (hf + 1) * 128], ident
                            )
                            nc.scalar.copy(ot[:, hf * H : (hf + 1) * H], tp)
                        y_ps = ps_y.tile([H, C], FP, name="y_e", tag="y")
                        nc.tensor.matmul(
                            y_ps, ot[:, 0:H], wpt_sb[:, 0:C],
                            start=True, stop=False,
                        )
                        nc.tensor.matmul(
                            y_ps, ot[:, H : 2 * H], wpt_sb[:, C : 2 * C],
                            start=False, stop=False,
                        )
                        nc.tensor.matmul(
                            y_ps, ones1, bp_sb, start=False, stop=True
                        )
                        ye = edgeo.tile([H, C], BF, name="ye")
                        nc.scalar.copy(ye, y_ps)
                        nc.sync.dma_start(
                            o3[3 : H - 3, j0, :], ye[3 : H - 3, :]
                        )

            def emit_corners():
                cornp, edgeo = state["cornp"], state["edgeo"]
                vev = v_edge.rearrange("p (cc c) -> p cc c", cc=14)
                o3 = out_d[:].rearrange("(i j) c -> i j c", j=W)
                acorn_sb = cornp.tile([KK, 36 * NH], BF, name="acorn")
                nc.sync.dma_start(acorn_sb, acorn_d[:])
                for blk in range(4):
                    ib, jb = blk // 2, blk % 2
                    si_c = 0 if ib == 0 else H - K
                    ccb = 0 if jb == 0 else 7
                    # all 9 pixels of a corner block share one clamped
                    # 7x7 window
                    vp = cornp.tile([KK, C], BF, name="vp")
                    nc.sync.dma_start(
                        vp, vev[si_c : si_c + K, ccb : ccb + 7, :]
                    )
                    co = edgeo.tile([9, C], BF, name="co")
                    # unpermute the window to ORIGINAL channel order so the
                    # per-head matmuls read/write contiguous slices
                    vp_std = cornp.tile([KK, C], BF, name="vp_std")
                    nc.vector.tensor_copy(
                        vp_std.rearrange("p (h d2 two) -> p h d2 two", h=NH, d2=16),
                        vp.rearrange("p (d2 h two) -> p h d2 two", d2=16, h=NH),
                    )
                    for qq in range(9):
                        q = blk * 9 + qq
                        c_ps = ps_a.tile([1, C], FP, name="c_ps", tag="acc")
                        for h in range(NH):
                            nc.tensor.matmul(
                                c_ps[:, h * HD : (h + 1) * HD],
                                acorn_sb[:, q * NH + h : q * NH + h + 1],
                                vp_std[:, h * HD : (h + 1) * HD],
                                start=True,
                                stop=True,
                            )
                        cs = edgeo.tile([1, C], BF, name="cs")
                        nc.scalar.copy(cs, c_ps)
                        nc.sync.dma_start(co[qq : qq + 1, :], cs)
                    # project the 9 corner pixels
                    cot = edgeo.tile([128, 2 * 9], BF, name="cot")
                    for hf in range(2):
                        tp9 = ps_t.tile([128, 9], BF, name="tp9", tag="tp")
                        nc.tensor.transpose(
                            tp9, co[:, hf * 128 : (hf + 1) * 128],
                            ident[0:9, 0:9],
                        )
                        nc.scalar.copy(cot[:, hf * 9 : (hf + 1) * 9], tp9)
                    y9 = ps_y.tile([9, C], FP, name="y9", tag="y")
                    nc.tensor.matmul(
                        y9, cot[:, 0:9], wpt0_sb[:, 0:C], start=True, stop=False
                    )
                    nc.tensor.matmul(
                        y9, cot[:, 9:18], wpt0_sb[:, C : 2 * C],
                        start=False, stop=False,
                    )
                    nc.tensor.matmul(
                        y9, ones1[0:1, 0:9], bp_sb, start=False, stop=True
                    )
                    y9s = edgeo.tile([9, C], BF, name="y9s")
                    nc.scalar.copy(y9s, y9)
                    i0 = 0 if ib == 0 else H - 3
                    j0 = 0 if jb == 0 else W - 3
                    nc.sync.dma_start(o3[i0 : i0 + 3, j0 : j0 + 3, :], y9s)

            # ---------------- emission schedule ----------------
            with (
                tc.tile_pool(name="ringp", bufs=1) as ringp,
                tc.tile_pool(name="xrp", bufs=2) as xrp,
                tc.tile_pool(name="xtp", bufs=2) as xtp,
                tc.tile_pool(name="vsp", bufs=2) as vsp,
                tc.tile_pool(name="attnp", bufs=4) as attnp,
                tc.tile_pool(name="prodp", bufs=6) as prodp,
                tc.tile_pool(name="outp", bufs=2) as outp,
                tc.tile_pool(name="otp", bufs=2) as otp,
                tc.tile_pool(name="y4p", bufs=2) as y4p,
            ):
                v_ring = ringp.tile([128, K * 2 * R * C], BF)
                nc.vector.memset(v_ring, 0.0)
                state.update(
                    # write view: doubled copies at flat slots s and s+R
                    vrw=v_ring.rearrange(
                        "p (kj d sc) -> p kj d sc", kj=K, d=2
                    ),
                    # read view: flat 2R slots, window never wraps
                    vrr=v_ring.rearrange(
                        "p (kj s c) -> p kj s c", kj=K, s=2 * R
                    ),
                    xrp=xrp, xtp=xtp, vsp=vsp, attnp=attnp,
                    prodp=prodp, outp=outp, otp=otp, y4p=y4p,
                )
                for t0 in range(3):
                    prefetch_attn(t0 * G)
                    proj_load(t0 * G)
                    for g in range(G):
                        proj_row(t0 * G, g)
                    proj_ring(t0 * G, 0, G)
                for t in range(NG):
                    if t + 3 < NG:
                        prefetch_attn((t + 3) * G)
                        proj_load((t + 3) * G)
                        emit_agg_group(t * G, proj_r0=(t + 3) * G)
                    else:
                        emit_agg_group(t * G)
                flush_agg()
            tc.strict_bb_all_engine_barrier()
            with (
                tc.tile_pool(name="edgep", bufs=1) as edgep,
                tc.tile_pool(name="prodp2", bufs=2) as prodp2,
                tc.tile_pool(name="edgeo", bufs=2) as edgeo,
                tc.tile_pool(name="cornp", bufs=2) as cornp,
            ):
                state.update(
                    ae=edgep.tile([H, 6 * KK * NH * 2], BF, name="ae"),
                    vew=edgep.tile([H, K * K * C], BF, name="vew"),
                    prodp2=prodp2, edgeo=edgeo, cornp=cornp,
                )
                nc.vector.memset(state["vew"], 0.0)
                emit_edges()
                emit_corners()

    if not nc.is_finalized():
        nc.finalize()
    return nc


# ---------------------- host side ----------------------

from ml_dtypes import bfloat16 as _bf16  # noqa: E402

# channel permutation: c' = d2*16 + h*2 + pair  <->  c = h*32 + d2*2 + pair
_PERM = np.empty(C, np.int64)
for _cp in range(C):
    _d2, _rem = _cp // 16, _cp % 16
    _h, _pair = _rem // 2, _rem % 2
    _PERM[_cp] = _h * 32 + _d2 * 2 + _pair
_PERM_H = _PERM // HD  # head of permuted channel c'


def _prep_attn(ab: np.ndarray, H: int, W: int):
    """attn[b] [NH,H,W,49] fp32 -> ([H*W, 784] bf16, [H, 6*784] bf16,
    [4*49, 9*256] bf16)."""
    a = ab.astype(_bf16)
    a6 = a.reshape(NH, H, W, K, K)  # [h, i, j, ki, kj]
    # interior: [i, j, kj, ki, h] -> dup pair
    ai = np.ascontiguousarray(a6.transpose(1, 2, 4, 3, 0))
    attnp = np.empty((H, W, K, K, NH, 2), _bf16)
    attnp[..., 0] = ai
    attnp[..., 1] = ai
    attnp = attnp.reshape(H * W, K * K * NH * 2)
    # edges: q = side*3+col -> [i, q, ki, cc, h] dup pair
    aedge = np.empty((H, 6, K, K, NH, 2), _bf16)
    for side in range(2):
        for col in range(3):
            j0 = col if side == 0 else W - 3 + col
            e = a6[:, :, j0, :, :].transpose(1, 2, 3, 0)  # [i, ki, cc, h]
            aedge[:, side * 3 + col, :, :, :, 0] = e
            aedge[:, side * 3 + col, :, :, :, 1] = e
    aedge = aedge.reshape(H, 6 * KK * NH * 2)
    # corners: [49, 36*NH] per-pixel attention vectors (original head idx)
    acorn = np.empty((KK, 36 * NH), _bf16)
    q = 0
    for ib in (0, 1):
        for jb in (0, 1):
            for ii in range(3):
                i0 = ii if ib == 0 else H - 3 + ii
                for jj in range(3):
                    j0 = jj if jb == 0 else W - 3 + jj
                    acorn[:, q * NH : (q + 1) * NH] = a[:, i0, j0, :].T
                    q += 1
    return attnp, aedge, acorn


_NC_CACHE: dict = {}


def _get_nc(H: int, W: int) -> bass.Bass:
    key = (H, W)
    if key not in _NC_CACHE:
        _NC_CACHE[key] = build_nc(H, W)
    return _NC_CACHE[key]


def make_in_maps(x, attn, Wv, bv, Wp, bp):
    x = np.asarray(x, np.float32)
    attn = np.asarray(attn, np.float32)
    B, H, W, C_ = x.shape
    assert C_ == C
    wvt = np.asarray(Wv, np.float32).T[:, _PERM].astype(_bf16)
    wvt = np.ascontiguousarray(wvt)
    wpt = np.asarray(Wp, np.float32).T[_PERM, :].astype(_bf16)
    wpt = np.ascontiguousarray(wpt)
    wpt0 = np.ascontiguousarray(np.asarray(Wp, np.float32).T.astype(_bf16))
    bv2 = np.asarray(bv, np.float32)[_PERM].reshape(1, C).astype(_bf16)
    bp2 = np.asarray(bp, np.float32).reshape(1, C).astype(_bf16)
    in_maps = []
    for b in range(B):
        xbf = x[b].reshape(H * W, C).astype(_bf16)
        attnp, aedge, acorn = _prep_attn(attn[b], H, W)
        in_maps.append(
            {
                "xbf": xbf,
                "attnp": attnp,
                "aedge": aedge,
                "acorn": acorn,
                "wvt": wvt,
                "bv": bv2,
                "wpt": wpt,
                "wpt0": wpt0,
                "bp": bp2,
            }
        )
    return in_maps


def kernel(x, attn, Wv, bv, Wp, bp):
    x = np.asarray(x, np.float32)
    B, H, W, C_ = x.shape
    nc = _get_nc(H, W)
    in_maps = make_in_maps(x, attn, Wv, bv, Wp, bp)
    from concourse.bass_utils import run_bass_kernel_spmd

    res = run_bass_kernel_spmd(nc, in_maps, list(range(B)))
    out = np.stack(
        [
            np.asarray(res.results[b]["out"]).astype(np.float32).reshape(H, W, C_)
            for b in range(B)
        ]
    )
    return out


if __name__ == "__main__":
    nc = build_nc()
    print("built OK")
